# revision 54
# baseline (speedup 1.0000x reference)
"""Trainium2 Bass kernel for nn_AttenLayer (ragged-sequence attention pooling).

Math (per batch b, with length L_b):
    proj   = tanh(nn_outs @ W^T + b)           # (S, A)
    scores = proj @ context                     # (S,)
    atten  = masked_softmax(scores, L_b)        # (S,), zeros beyond L_b
    out    = atten @ nn_outs                    # (H,)

Sharding: pure data-parallel over batch; 8 batch slots per core on 8 cores.

Raggedness: batches are sorted by L descending and dealt into 8 slots x
8 cores so that every core gets the identical slot profile prof_ex[0..7]
(max L of the 8 batches sharing the slot). The single SPMD program only
processes ~prof_ex[slot] steps per slot (exact widths for compute,
128-aligned extents for DMA/transposes), skipping ~1/3 of the full-S
work. The profile is data-dependent, so the Bass program is (re)built
per profile and cached.

Per-core plan (matmuls bf16, f32 PSUM accumulation):
  - nn_outs (host-cast bf16) is loaded twice per batch, valid chunks only:
      natural [s128, (n h)] tiles           (rhs of phase-3, k=s)
      host-transposed [h128, s] tiles       (rhs of phase-1, k=h)
  - phase 1: projT[a128, s<=512] psum = sum_h W^T[h,a] @ xT[h,s]
    in 512-step column chunks (last chunk of a slot may be partial);
    tanh+bias on ACT -> projT sbuf (bf16)
  - scores accumulate into per-wave [4, 512] psum tiles via zero-padded
    ctx trick (row bwi of the psum gets batch bwi's scores). The scores
    matmul for iteration (j,a) is traced two iterations late so the
    in-order PE queue never head-blocks on the ACT tanh.
  - two waves of 4 slots; per-wave masked softmax on [4, W] where
    W = exact max L in the wave (additive -30000 mask; exp on ACT with
    fused accumulate; atten scaled by 1/denom on ACT); out DMAs ride the
    ACT queue so the SP queue keeps prefetching ahead
  - atten [4,128] chunks transposed on PE -> attT [128, (jj,bwi)] bf16
  - phase 3: out[1, h512] psum = sum_s attT[s,1] @ nat[s,h]
  - wave w's softmax/phase-3 are interleaved into wave w+1's phase-1
    trace so no engine head-blocks across the wave boundary.
"""

import sys

for _p in ("/opt/trn_rl_repo",):
    if _p not in sys.path:
        sys.path.insert(0, _p)

import numpy as np
import ml_dtypes

import concourse.bass as bass
from concourse import bacc
import concourse.mybir as mybir
import concourse.tile as tile
from concourse.masks import make_identity

B, S, H, A = 64, 2048, 512, 512
NCORES = 8
BPC = B // NCORES          # batch slots per core
WB = 4                     # wave size (and ctx zero-pad layout width)
NW = BPC // WB             # number of waves

SK = S // 128              # max 128-step chunks per batch
AC = A // 128
HC = H // 128

F32 = mybir.dt.float32
BF16 = mybir.dt.bfloat16


def build_nc(prof_ex, repeat: int = 1) -> bass.Bass:
    """prof_ex: tuple of 8 ints (desc), max valid steps (exact L) per slot."""
    assert len(prof_ex) == BPC and all(1 <= ex <= S for ex in prof_ex)
    assert all(prof_ex[i] >= prof_ex[i + 1] for i in range(BPC - 1))
    prof128 = [(ex + 127) // 128 for ex in prof_ex]
    nc = bacc.Bacc()

    x_bf = nc.declare_dram_parameter("x_bf", [BPC, S, H], BF16, isOutput=False)
    xt_d = nc.declare_dram_parameter("xt_d", [BPC, H, S], BF16, isOutput=False)
    # W^T pre-chunked on host: wt[p, c*A + a] = proj_w[a, 128c + p]
    wt_d = nc.declare_dram_parameter("wt", [128, HC * A], BF16, isOutput=False)
    ctx_d = nc.declare_dram_parameter("ctx", [128, AC * WB * WB], BF16, isOutput=False)
    pb_d = nc.declare_dram_parameter("pb", [128, AC], F32, isOutput=False)
    mask_d = nc.declare_dram_parameter("mask", [BPC, S], BF16, isOutput=False)
    out_d = nc.declare_dram_parameter("out", [BPC, H], F32, isOutput=True)

    # per-slot: number of 512-col phase-1 chunks and per-chunk widths.
    # All compute (p1 matmul, tanh, scores, softmax) runs at the exact
    # per-slot width; only DMA extents and the transpose chunks are
    # 128-aligned (the `at` pad columns are zeroed once so transposes
    # read initialized data).
    pj = [(q * 128 + 511) // 512 for q in prof128]     # == ceil(ex/512)
    def width(slot, j):
        return min(512, prof_ex[slot] - 512 * j)
    # waves (start_slot, n_slots): lead wave heavy, last wave light so the
    # un-overlapped tail (last wave's softmax + phase 3) stays small
    WAVES = [(0, 4), (4, 4)]
    NWV = len(WAVES)
    # first slot of a wave has the max q in the wave (profile is desc)
    pmaxs = [pj[s0] for s0, _ in WAVES]
    wwidth = [prof_ex[s0] for s0, _ in WAVES]          # softmax window W
    wwidth128 = [prof128[s0] * 128 for s0, _ in WAVES]
    # per (wave, j): number of contributing slots (prefix of the wave)
    cnt = {}
    for w, (s0, wb) in enumerate(WAVES):
        for j in range(pmaxs[w]):
            cnt[(w, j)] = sum(1 for si in range(wb) if pj[s0 + si] > j)

    with tile.TileContext(nc) as tc:
        with (
            tc.tile_pool(name="const", bufs=1) as const_pool,
            tc.tile_pool(name="nat", bufs=BPC) as nat_pool,
            tc.tile_pool(name="xt", bufs=3) as xt_pool,
            tc.tile_pool(name="projT", bufs=6) as proj_pool,
            tc.tile_pool(name="smx", bufs=2) as smx_pool,
            tc.tile_pool(name="attT", bufs=2) as attT_pool,
            tc.tile_pool(name="osb", bufs=4) as os_pool,
            tc.tile_pool(name="p1ps", bufs=2, space="PSUM") as p1_psum,
            tc.tile_pool(name="scps", bufs=4, space="PSUM") as sc_psum,
            tc.tile_pool(name="atps", bufs=1, space="PSUM") as at_psum,
            tc.tile_pool(name="ops", bufs=1, space="PSUM") as out_psum,
        ):
            wt_sb = const_pool.tile([128, HC * A], BF16, tag="wt")
            ctx_sb = const_pool.tile([128, AC * WB * WB], BF16, tag="ctx")
            pb_sb = const_pool.tile([128, AC], F32, tag="pb")
            mask_w = {}
            ident = const_pool.tile([128, 128], F32, tag="ident")
            make_identity(nc, ident[:])
            consts_loaded = [False]

            def setup_masks():
                # pre-issue all mask DMAs so softmax never waits on them
                for w, (s0, wb) in enumerate(WAVES):
                    mw = const_pool.tile(
                        [wb, wwidth[w]], BF16, tag=f"mask{w}", name="mask"
                    )
                    nc.sync.dma_start(mw[:], mask_d[s0 : s0 + wb, : wwidth[w]])
                    mask_w[w] = mw

            def pe_warmup():
                # dummy transposes keep PE busy (and its p-state ramping)
                # while the first xt/wt DMAs are in flight
                ps = p1_psum.tile([128, 512], F32, tag="p1", name="warm")
                for _i in range(10):
                    nc.tensor.transpose(ps[:, :128], ident[:], ident[:])

            nat = {}    # slot -> natural tile [128, (n, h)]
            opss = {}   # w -> phase-3 psum bank [128, 512]
            scpss = {}  # w -> list of pmax scores psum tiles [WB, 512]
            attT = {}   # (w, g) -> [128, 4*WB] bf16, col = WB*jj + bwi

            def trace_batch(w, si):
                slot = WAVES[w][0] + si
                q = prof128[slot]
                # one xt tile per slot, layout [p, (hc, s)]; a single DMA
                # (or one per j-chunk for slot 0) loads all four h-chunks
                xt = xt_pool.tile([128, HC * 128 * q], BF16, tag="xt", name="xt")
                xt3 = xt[:].rearrange("p (hc s) -> p hc s", hc=HC)
                src3 = xt_d[slot, :, : 128 * q].rearrange("(hc p) s -> p hc s", p=128)
                if slot == 0 and not consts_loaded[0]:
                    # wt first (one DMA), then xt j-chunked so the first
                    # matmuls start after ~0.8MB of DMA instead of the
                    # whole batch
                    nc.sync.dma_start(wt_sb[:], wt_d[:])
                    for j in range(pj[slot]):
                        jsl = slice(512 * j, 512 * j + width(slot, j))
                        nc.sync.dma_start(xt3[:, :, jsl], src3[:, :, jsl])
                        if j == 0:
                            nc.sync.dma_start(pb_sb[:], pb_d[:])
                            nc.sync.dma_start(ctx_sb[:], ctx_d[:])
                    consts_loaded[0] = True
                else:
                    nc.sync.dma_start(xt3[:], src3[:])
                natb = nat_pool.tile(
                    [128, q * 512], BF16, tag=f"nat{slot}", name="nat", bufs=1
                )
                nat[slot] = natb
                nc.sync.dma_start(
                    natb[:].rearrange("p (n h) -> p n h", n=q),
                    x_bf[slot, : 128 * q].rearrange("(n p) h -> p n h", p=128),
                )
                # phase 1 with the scores matmul lagging one (j,a) iteration
                # so PE never waits on the ACT tanh.
                pending = []  # [(j, a, n, projT tile)] lag-2 queue
                for j in range(pj[slot]):
                    n = width(slot, j)
                    for a in range(AC):
                        ps = p1_psum.tile([128, 512], F32, tag="p1")
                        for hc in range(HC):
                            off = hc * 128 * q + 512 * j
                            nc.tensor.matmul(
                                ps[:, :n],
                                wt_sb[:, hc * A + a * 128 : hc * A + (a + 1) * 128],
                                xt[:, off : off + n],
                                start=(hc == 0),
                                stop=(hc == HC - 1),
                            )
                        pt = proj_pool.tile([128, 512], BF16, tag="projT")
                        nc.scalar.activation(
                            pt[:, :n],
                            ps[:, :n],
                            mybir.ActivationFunctionType.Tanh,
                            bias=pb_sb[:, a : a + 1],
                        )
                        if len(pending) >= 2:
                            emit_scores(w, si, *pending.pop(0))
                        pending.append((j, a, n, pt))
                for unit in pending:
                    emit_scores(w, si, *unit)

            def emit_scores(w, si, j, a, n, pt):
                # ctx col block (a, si): only column si is context's a-chunk,
                # rest zero, so only row si of the wave's scores psum
                # accumulates this batch. The wave's first slot has its max
                # width, so its start=True zeroes every region later slots
                # touch.
                wb = WAVES[w][1]
                nc.tensor.matmul(
                    scpss[w][j][:, :n],
                    ctx_sb[:, (a * WB + si) * WB : (a * WB + si) * WB + wb],
                    pt[:, :n],
                    start=(si == 0 and a == 0),
                    stop=(si == cnt[(w, j)] - 1 and a == AC - 1),
                )

            def softmax_dve(w):
                # DVE part: additive mask, running chunk maxes, final -max
                W = wwidth[w]
                wb = WAVES[w][1]
                scm = smx_pool.tile([wb, W], F32, tag=f"scm{w}", name="scm", bufs=1)
                pmx = smx_pool.tile([wb, pmaxs[w]], F32, tag="pmx")
                for j in range(pmaxs[w]):
                    n = min(512, W - 512 * j)
                    sl = slice(512 * j, 512 * j + n)
                    nc.vector.tensor_tensor(
                        out=scm[:, sl], in0=scpss[w][j][:, :n],
                        in1=mask_w[w][:, sl], op=mybir.AluOpType.add,
                    )
                    nc.vector.reduce_max(
                        pmx[:, j : j + 1], scm[:, sl], axis=mybir.AxisListType.X
                    )
                mx = smx_pool.tile([wb, 1], F32, tag="mx")
                nc.vector.reduce_max(
                    mx[:], pmx[:], axis=mybir.AxisListType.X, negate=True
                )
                return scm, mx

            def softmax_act(w, scm, mx):
                # ACT/PE part: exp (+denominator), scale, transpose chunks
                W = wwidth[w]
                W128 = wwidth128[w]
                wb = WAVES[w][1]
                ex = smx_pool.tile([wb, W], BF16, tag=f"ex{w}", name="ex", bufs=1)
                rs = smx_pool.tile([wb, 1], F32, tag="rs")
                nc.scalar.activation(
                    ex[:],
                    scm[:],
                    mybir.ActivationFunctionType.Exp,
                    bias=mx[:],
                    accum_out=rs[:],
                )
                rv = smx_pool.tile([wb, 1], F32, tag="rv")
                nc.vector.reciprocal(rv[:], rs[:])
                # at is 128-aligned for the transposes; the pad columns are
                # zeroed once (bufs=1: they stay zero across reps)
                at = smx_pool.tile([wb, W128], F32, tag=f"at{w}", name="at", bufs=1)
                if W128 > W:
                    nc.vector.memset(at[:, W:], 0.0)
                nc.scalar.activation(
                    at[:, :W], ex[:], mybir.ActivationFunctionType.Copy,
                    scale=rv[:],
                )
                q0 = prof128[WAVES[w][0]]
                for g in range((q0 + 3) // 4):
                    njj = min(4, q0 - 4 * g)
                    aps = at_psum.tile([128, 4 * wb], F32, tag="atps")
                    for jj in range(njj):
                        k = 4 * g + jj
                        nc.tensor.transpose(
                            aps[:, jj * wb : (jj + 1) * wb],
                            at[:, k * 128 : (k + 1) * 128],
                            ident[:wb, :wb],
                        )
                    att_sb = attT_pool.tile([128, 4 * wb], BF16, tag=f"attT{g}")
                    # copy only the transposed columns; stale psum beyond
                    # njj*wb is never read by phase 3
                    nc.vector.tensor_copy(
                        att_sb[:, : njj * wb], aps[:, : njj * wb]
                    )
                    attT[(w, g)] = att_sb

            def phase3_batch(w, bwi):
                s0, wb = WAVES[w]
                slot = s0 + bwi
                q = prof128[slot]
                # batches of a wave accumulate in ONE psum bank at
                # partition bases 0/32/64 (the legal bases), so up to three
                # phase-3 groups run concurrently without bank serialization
                base = 32 * (bwi % 3)
                ops = opss[w][base : base + 1, :]
                for k in range(q):
                    col = (k % 4) * wb + bwi
                    nc.tensor.matmul(
                        ops,
                        attT[(w, k // 4)][:, col : col + 1],
                        nat[slot][:, k * 512 : (k + 1) * 512],
                        start=(k == 0),
                        stop=(k == q - 1),
                    )
                os_b = os_pool.tile([1, H], F32, tag="os")
                nc.vector.tensor_copy(os_b[:], ops)
                # out DMA on the ACT queue: keeps SP free to run ahead and
                # prefetch the next slots' (and next rep's) xt/nat loads
                nc.scalar.dma_start(out_d[slot : slot + 1, :], os_b[:])

            for _rep in range(repeat):
                nat.clear(); attT.clear(); opss.clear()
                if _rep == 0:
                    pe_warmup()
                carry = None  # previous wave awaiting softmax/phase-3
                for w, (s0, wb) in enumerate(WAVES):
                    if carry is not None:
                        scm, mx = softmax_dve(carry)
                    scpss[w] = [
                        sc_psum.tile([wb, 512], F32, tag="scps", name="scps")
                        for _j in range(pmaxs[w])
                    ]
                    pending3 = []  # phase-3 units of the carried wave
                    trace_batch(w, 0)
                    if _rep == 0 and w == 0:
                        setup_masks()
                    if carry is not None:
                        softmax_act(carry, scm, mx)
                        opss[carry] = out_psum.tile(
                            [128, 512], F32, tag="ops", name="ops"
                        )
                        pending3 = [(carry, b) for b in range(WAVES[carry][1])]
                    for si in range(1, wb):
                        trace_batch(w, si)
                        if pending3:
                            phase3_batch(*pending3.pop(0))
                    for unit in pending3:
                        phase3_batch(*unit)
                    carry = w
                scm, mx = softmax_dve(carry)
                softmax_act(carry, scm, mx)
                opss[carry] = out_psum.tile(
                    [128, 512], F32, tag="ops", name="ops"
                )
                for bwi in range(WAVES[carry][1]):
                    phase3_batch(carry, bwi)

    nc.finalize()
    return nc


_NC_CACHE = {}


def get_nc(prof_ex, repeat: int = 1) -> bass.Bass:
    key = (tuple(prof_ex), repeat)
    if key not in _NC_CACHE:
        _NC_CACHE[key] = build_nc(tuple(prof_ex), repeat=repeat)
    return _NC_CACHE[key]


def plan(batch_lens):
    """Slot profile + batch->(core, slot) assignment (identical per core)."""
    lens = np.asarray(batch_lens).reshape(B).astype(np.int64)
    order = np.argsort(-lens, kind="stable")  # batches desc by length
    prof_ex = tuple(int(lens[order[BPC * i]]) for i in range(BPC))
    # core cc, slot i <- batch order[8i + cc]; every core sees `prof_ex`
    batch_of = np.empty((NCORES, BPC), np.int64)
    for i in range(BPC):
        for cc in range(NCORES):
            batch_of[cc, i] = order[BPC * i + cc]
    return prof_ex, batch_of, lens


def make_in_maps(nn_outs, batch_lens, context, proj_w, proj_b):
    """Host-side shard prep. Returns (profile, per-core inputs, batch_of)."""
    prof128, batch_of, lens = plan(batch_lens)
    x_bf = np.asarray(nn_outs, dtype=np.float32).astype(ml_dtypes.bfloat16)
    xt_host = np.ascontiguousarray(x_bf.transpose(0, 2, 1))  # [B, H, S]
    wt = np.ascontiguousarray(np.asarray(proj_w, np.float32).T)  # [H, A]
    # wt_sb[p, c*A + a] = wt[128c + p, a]
    wt_host = np.ascontiguousarray(
        wt.reshape(HC, 128, A).transpose(1, 0, 2).reshape(128, HC * A)
    ).astype(ml_dtypes.bfloat16)
    ctx_c = np.asarray(context, np.float32).reshape(AC, 128)
    ctx_host = np.zeros((128, AC, WB, WB), np.float32)
    for a in range(AC):
        for bw in range(WB):
            ctx_host[:, a, bw, bw] = ctx_c[a]
    ctx_host = np.ascontiguousarray(
        ctx_host.reshape(128, AC * WB * WB)
    ).astype(ml_dtypes.bfloat16)
    pb_host = np.ascontiguousarray(
        np.asarray(proj_b, np.float32).reshape(AC, 128).T
    )
    iota = np.arange(S)[None, :]
    mask_add = np.where(iota < lens[:, None], 0.0, -30000.0).astype(
        ml_dtypes.bfloat16
    )
    in_maps = []
    for cc in range(NCORES):
        sel = batch_of[cc]
        in_maps.append(
            {
                "x_bf": np.ascontiguousarray(x_bf[sel]),
                "xt_d": np.ascontiguousarray(xt_host[sel]),
                "wt": wt_host,
                "ctx": ctx_host,
                "pb": pb_host,
                "mask": np.ascontiguousarray(mask_add[sel]),
            }
        )
    return prof128, in_maps, batch_of


def run(nn_outs, batch_lens, context, proj_w, proj_b, trace=False,
        repeat: int = 1, **trace_kw):
    from concourse.bass_utils import run_bass_kernel_spmd

    prof128, in_maps, batch_of = make_in_maps(
        nn_outs, batch_lens, context, proj_w, proj_b
    )
    nc = get_nc(prof128, repeat=repeat)
    res = run_bass_kernel_spmd(
        nc, in_maps, list(range(NCORES)), trace=trace, **trace_kw
    )
    out = np.empty((B, H), np.float32)
    for cc in range(NCORES):
        out[batch_of[cc]] = res.results[cc]["out"]
    return out, res


def kernel(nn_outs, batch_lens, context, proj_w, proj_b):
    out, _ = run(nn_outs, batch_lens, context, proj_w, proj_b, trace=False)
    return out
